# revision 12
# baseline (speedup 1.0000x reference)
"""TRN2 Bass kernel for nn_CausalLoopJIT (echo-state reservoir pair + degenerate GRU).

Strategy (pure data-parallel, batch sharded 8 ways, 32 lanes/core):
- All recurrent state kept feature-major on SBUF partitions; batch rides the
  matmul free dim.  Big weights stream as rhs in float32r (1 cyc/row at N>=256).
- float32r is tf32-like (~11-bit mantissa), so every f32r matmul is computed as
  a hi/lo split (x_h*W_h + x_l*W_h + x_h*W_l) restoring ~2^-22 precision.
- The GRU leg (gx -> h -> logits) is pair-batched across 2 consecutive steps
  (the D=2 delay line gives exactly that much slack), halving its stream cost.
- sigmoid(-x) = (1 - tanh(x/2))/2 rewrite (0.5 pre-folded into weights) makes
  every transcendental a Tanh: one ACT table set, one big ACT op per group.
- Outputs: raw logits staged on-chip, one DMA at the end; host applies sign(),
  washout slice; settings_log is recomputed host-side from the input bits.
"""
import os
import numpy as np
from contextlib import ExitStack

from concourse import bass, bacc, tile, mybir
from concourse.bass_utils import run_bass_kernel_spmd

F32 = mybir.dt.float32
F32R = mybir.dt.float32r
AF = mybir.ActivationFunctionType

B, T, N, H, D = 256, 512, 256, 256, 2
NCORES = 8
BL = B // NCORES          # 32 batch lanes per core
PAIRS = T // 2

# f32r const blob column offsets
WZN_H, WZN_L = 0, 2048
WREC_H, WREC_L = 4096, 5120
TBL = 6144       # rows 0:8
WIN = 6656       # rows 32:46
ZR = 7168        # zeros [128, 128]
CR_TOT = 7296
# f32 const blob
WOUT = 0         # [128, 4]: chunk k at cols 2k:2k+2
IDENT = 4        # rows 0:64, cols 4:68
ZF = 68          # zeros [128, 128]
CF_TOT = 196


def _round10(x):
    """Round fp32 to 10 explicit mantissa bits (RNE) - exactly representable in f32r."""
    u = np.asarray(x, np.float32).view(np.uint32)
    r = (u + np.uint32(0xFFF) + ((u >> np.uint32(13)) & np.uint32(1))) & np.uint32(0xFFFFE000)
    return r.view(np.float32).copy()


def _split(x):
    h = _round10(x)
    l = _round10(np.asarray(x, np.float32) - h)
    return h, l


def build_bass(Tsteps=T, split=True):
    """Build the per-core Bass program (SPMD; same program all 8 cores)."""
    pairs = Tsteps // 2
    nc = bacc.Bacc("TRN2", target_bir_lowering=False, debug=False, num_devices=NCORES)
    cr_d = nc.dram_tensor("cr", [128, CR_TOT], F32R, kind="ExternalInput").ap()
    cf_d = nc.dram_tensor("cf", [128, CF_TOT], F32, kind="ExternalInput").ap()
    noi_d = nc.dram_tensor("noi", [pairs, 128, 256], F32, kind="ExternalInput").ap()
    ohp_d = nc.dram_tensor("ohp", [pairs, 8, 64], F32R, kind="ExternalInput").ap()
    smp_d = nc.dram_tensor("smp", [pairs, 8, 64], F32R, kind="ExternalInput").ap()
    lgo_d = nc.dram_tensor("lgo", [2, pairs * 64], F32, kind="ExternalOutput").ap()

    with tile.TileContext(nc) as tc, ExitStack() as ctx:
        cst = ctx.enter_context(tc.tile_pool(name="cst", bufs=1))
        pn = ctx.enter_context(tc.tile_pool(name="pn", bufs=3))
        po = ctx.enter_context(tc.tile_pool(name="po", bufs=3))
        psm = ctx.enter_context(tc.tile_pool(name="psm", bufs=6))
        pst = ctx.enter_context(tc.tile_pool(name="pst", bufs=3))
        ppr = ctx.enter_context(tc.tile_pool(name="ppr", bufs=2))
        pgxp = ctx.enter_context(tc.tile_pool(name="pgxp", bufs=2, space="PSUM"))
        presp = ctx.enter_context(tc.tile_pool(name="presp", bufs=2, space="PSUM"))
        ptrp = ctx.enter_context(tc.tile_pool(name="ptrp", bufs=2, space="PSUM"))
        ptr2p = ctx.enter_context(tc.tile_pool(name="ptr2p", bufs=2, space="PSUM"))
        pdr = ctx.enter_context(tc.tile_pool(name="pdr", bufs=3, space="DRAM"))

        cr = cst.tile([128, CR_TOT], F32R)
        cf = cst.tile([128, CF_TOT], F32)
        stage = cst.tile([2, pairs * 64], F32)
        nc.sync.dma_start(cr[:], cr_d[:])
        nc.sync.dma_start(cf[:], cf_d[:])

        def wzn(part, c):
            return cr[:, part + 512 * c: part + 512 * (c + 1)]

        def wrec(part, c):
            return cr[:, part + 256 * c: part + 256 * (c + 1)]

        zeros_r = cr[:, ZR:ZR + 128]      # f32r zeros [128,128]
        ident = cf[:, IDENT:IDENT + 64]   # rows 0:64 used

        # per-pair input tiles (DMA'd with lookahead)
        noi_t, oh_t, sm_t = {}, {}, {}

        def emit_dmas(q):
            if q >= pairs:
                return
            noi_t[q] = pn.tile([128, 256], F32, tag="noi", name="noi")
            oh_t[q] = po.tile([8, 64], F32R, tag="oh", name="oh")
            nc.sync.dma_start(noi_t[q][:], noi_d[q])
            nc.sync.dma_start(ohp_t := oh_t[q][:], ohp_d[q])
            for j in (0, 1):
                sm_t[2 * q + j] = psm.tile([46, 32], F32R, tag="sm", name="sm")
                nc.sync.dma_start(sm_t[2 * q + j][38:46, :], smp_d[q][:, 32 * j:32 * (j + 1)])

        emit_dmas(0)
        emit_dmas(1)
        # step-0/1 del rows start at zero (delay buffer init)
        nc.sync.dma_start(sm_t[0][32:38, :], cr_d[32:38, ZR:ZR + 32])
        nc.sync.dma_start(sm_t[1][32:38, :], cr_d[32:38, ZR + 32:ZR + 64])

        # aab(0) slot 0 = 0 + noise(0)  (stored interleaved: slot j at cols 32j of each 64)
        aab_h = {0: pst.tile([128, 4, 64], F32R, tag="aabh", name="aabh")}
        aab_l = {0: pst.tile([128, 4, 64], F32R, tag="aabl", name="aabl")}
        nv0 = noi_t[0][:, 0:128].rearrange("p (c b) -> p c b", c=4)
        nc.vector.tensor_copy(aab_h[0][:, :, 0:32], nv0)
        if split:
            nc.vector.tensor_sub(aab_l[0][:, :, 0:32], nv0, aab_h[0][:, :, 0:32])

        sab_h = {-1: None}
        sab_l = {-1: None}
        ptr_tiles = {}

        def res_step(t):
            """Reservoir matmuls for step t -> pres tile [32, 512] (A cols 0:256, B 256:512)."""
            q, j = t // 2, t % 2
            pres = presp.tile([BL, 512], F32, tag="pres", name="pres")
            sh = sab_h[t - 1]
            sl = sab_l[t - 1]
            # win first: K=14 rows 32:46, covers both halves, opens the group
            nc.tensor.matmul(pres[:, 0:512], sm_t[t][32:46, :],
                             cr[32:46, WIN:WIN + 512], start=True, stop=False,
                             tile_position=(32, 0), skip_group_check=True)
            mms = []
            for c in range(4):
                lh = zeros_r[:, 32 * c:32 * (c + 1)] if sh is None else sh[:, c, :]
                mms.append((c, lh, WREC_H))
            if split:
                for c in range(4):
                    ll = zeros_r[:, 32 * c:32 * (c + 1)] if sl is None else sl[:, c, :]
                    lh = zeros_r[:, 32 * c:32 * (c + 1)] if sh is None else sh[:, c, :]
                    mms.append((c, ll, WREC_H))
                    mms.append((c, lh, WREC_L))
            for i, (c, lhs, wpart) in enumerate(mms):
                half = 0 if c < 2 else 1
                out = pres[:, 256 * half:256 * (half + 1)]
                nc.tensor.matmul(out, lhs, wrec(wpart, c), start=False,
                                 stop=(i == len(mms) - 1), skip_group_check=True)
            return pres

        def state_chain(t, pres):
            """tanh -> transpose -> split; also build aab slot for step t+1."""
            q, j = t // 2, t % 2
            sasb = ppr.tile([BL, 512], F32, tag="sasb", name="sasb")
            nc.scalar.activation(sasb[:], pres[:], AF.Tanh)
            ptr = ptrp.tile([128, 4, 32], F32, tag="ptr", name="ptr")
            for c in range(4):
                nc.tensor.transpose(ptr[:, c, :], sasb[:, 128 * c:128 * (c + 1)], ident[0:BL, 0:BL])
            ptr_tiles[t] = ptr
            sh = pst.tile([128, 4, 32], F32R, tag="sabh", name="sabh")
            nc.vector.tensor_copy(sh[:], ptr[:])
            sab_h[t] = sh
            if split:
                sl = pst.tile([128, 4, 32], F32R, tag="sabl", name="sabl")
                nc.vector.tensor_sub(sl[:], ptr[:], sh[:])
                sab_l[t] = sl
            else:
                sab_l[t] = None
            # aab for step t+1 (pair qn slot jn)
            tn_ = t + 1
            if tn_ >= Tsteps:
                return
            qn, jn = tn_ // 2, tn_ % 2
            if jn == 0:
                aab_h[qn] = pst.tile([128, 4, 64], F32R, tag="aabh", name="aabh")
                aab_l[qn] = pst.tile([128, 4, 64], F32R, tag="aabl", name="aabl")
            nv = noi_t[qn][:, 128 * jn:128 * (jn + 1)].rearrange("p (c b) -> p c b", c=4)
            tmp = ppr.tile([128, 4, 32], F32, tag="tmp", name="tmp")
            nc.vector.tensor_add(tmp[:], ptr[:], nv)
            nc.vector.tensor_copy(aab_h[qn][:, :, 32 * jn:32 * (jn + 1)], tmp[:])
            if split:
                nc.vector.tensor_sub(aab_l[qn][:, :, 32 * jn:32 * (jn + 1)], tmp[:],
                                     aab_h[qn][:, :, 32 * jn:32 * (jn + 1)])

        def gx_pair(q):
            """Paired GRU leg for steps (2q, 2q+1): gx, h', logits, del splits, staging."""
            pgx = pgxp.tile([64, 512], F32, tag="pgx", name="pgx")
            ah, al = aab_h[q], aab_l[q]
            for c in range(4):
                nc.tensor.matmul(pgx[:], ah[:, c, :], wzn(WZN_H, c), start=(c == 0), stop=False)
            if split:
                for c in range(4):
                    nc.tensor.matmul(pgx[:], al[:, c, :], wzn(WZN_H, c), start=False, stop=False)
                    nc.tensor.matmul(pgx[:], ah[:, c, :], wzn(WZN_L, c), start=False, stop=False)
            nc.tensor.matmul(pgx[:], oh_t[q][0:8, :], cr[0:8, TBL:TBL + 512],
                             start=False, stop=True)
            tzn = ppr.tile([64, 512], F32, tag="tzn", name="tzn")
            nc.scalar.activation(tzn[:], pgx[:], AF.Tanh)
            m = ppr.tile([64, 256], F32, tag="m", name="m")
            nc.vector.tensor_mul(m[:], tzn[:, 0:256], tzn[:, 256:512])
            hp = ppr.tile([64, 256], F32, tag="hp", name="hp")
            nc.vector.tensor_sub(hp[:], tzn[:, 256:512], m[:])
            ptr2 = ptr2p.tile([128, 192], F32, tag="ptr2", name="ptr2")
            for k in range(2):
                nc.tensor.transpose(ptr2[:, 64 * k:64 * (k + 1)], hp[:, 128 * k:128 * (k + 1)],
                                    ident[0:64, 0:64])
            hT = ppr.tile([128, 128], F32, tag="hT", name="hT")
            nc.vector.tensor_copy(hT[:], ptr2[:, 0:128])
            lg = ptr2[0:2, 128:192]
            for k in range(2):
                nc.tensor.matmul(lg, cf[:, WOUT + 2 * k:WOUT + 2 * (k + 1)],
                                 hT[:, 64 * k:64 * (k + 1)], start=(k == 0), stop=(k == 1))
            # staging (host applies sign + b_out)
            nc.vector.tensor_copy(stage[:, 64 * q:64 * (q + 1)], lg)
            # delayed-logit rows of smalls(q+1): rows 32:38 = dAh dAl dBh dBl dAh dBh.
            # Engine writes need 32-aligned base partitions, so split in a base-0
            # scratch [2,(h,l),64] then scatter rows via SBUF->SBUF DMAs.
            if q + 1 < pairs:
                sc = ppr.tile([2, 2, 64], F32R, tag="sc", name="sc")
                nc.vector.tensor_copy(sc[:, 0, :], lg)
                nc.vector.tensor_sub(sc[:, 1, :], lg, sc[:, 0, :])
                # bounce through DRAM: rows 0:6 = (dAh,dBh),(dAl,dBl),(dAh,dBh)
                scb = pdr.tile([8, 64], F32R, tag="scb", name="scb")
                nc.sync.dma_start(scb[0:2, :], sc[:, 0, :])
                nc.sync.dma_start(scb[2:4, :], sc[:, 1, :])
                nc.sync.dma_start(scb[4:6, :], sc[:, 0, :])
                for j in (0, 1):
                    s = sm_t[2 * (q + 1) + j]
                    nc.sync.dma_start(s[32:38, :], scb[0:6, 32 * j:32 * (j + 1)])

        for q in range(pairs):
            emit_dmas(q + 2)
            t0, t1 = 2 * q, 2 * q + 1
            pres0 = res_step(t0)
            state_chain(t0, pres0)
            pres1 = res_step(t1)
            state_chain(t1, pres1)
            gx_pair(q)
            # release old references
            for d in (sab_h, sab_l, ptr_tiles):
                d.pop(t0 - 2, None)
                d.pop(t1 - 2, None)
            aab_h.pop(q - 1, None)
            aab_l.pop(q - 1, None)
            noi_t.pop(q - 1, None)
            oh_t.pop(q - 1, None)
            sm_t.pop(2 * q - 2, None)
            sm_t.pop(2 * q - 1, None)

        nc.sync.dma_start(lgo_d[:], stage[:])
    nc.compile()
    return nc


def prep_consts(W_in_A, W_rec_A, W_in_B, W_rec_B, Wx, b_gru, Wout, b_out):
    """Host-side constant blobs (shared across cores)."""
    Wx = np.asarray(Wx, np.float64)
    # z columns scaled by 0.5 (tanh half-angle identity), n columns as-is
    Wzn = np.concatenate([Wx[:, 0:H] * 0.5, Wx[:, 2 * H:3 * H]], axis=1)  # [514, 512]
    Wzn_s, Wzn_spin = Wzn[0:512], Wzn[512:514]
    bzn = np.concatenate([np.asarray(b_gru, np.float64)[0:H] * 0.5,
                          np.asarray(b_gru, np.float64)[2 * H:3 * H]])
    cr = np.zeros((128, CR_TOT), np.float32)
    wh, wl = _split(Wzn_s.reshape(4, 128, 512))
    cr[:, WZN_H:WZN_H + 2048] = wh.transpose(1, 0, 2).reshape(128, 2048)
    cr[:, WZN_L:WZN_L + 2048] = wl.transpose(1, 0, 2).reshape(128, 2048)
    wrec = np.zeros((4, 128, 256), np.float64)
    wrec[0:2] = np.asarray(W_rec_A, np.float64).reshape(2, 128, 256)
    wrec[2:4] = np.asarray(W_rec_B, np.float64).reshape(2, 128, 256)
    rh, rl = _split(wrec)
    cr[:, WREC_H:WREC_H + 1024] = rh.transpose(1, 0, 2).reshape(128, 1024)
    cr[:, WREC_L:WREC_L + 1024] = rl.transpose(1, 0, 2).reshape(128, 1024)
    # table[idx] = bzn + (2*b0-1)*Wzn_spin[0] + (2*b1-1)*Wzn_spin[1], idx = 2*b0+b1
    tbl = np.zeros((4, 512), np.float64)
    for idx in range(4):
        b0, b1 = idx >> 1, idx & 1
        tbl[idx] = bzn + (2 * b0 - 1) * Wzn_spin[0] + (2 * b1 - 1) * Wzn_spin[1]
    th, tl = _split(tbl)
    cr[0:4, TBL:TBL + 512] = th
    cr[4:8, TBL:TBL + 512] = tl
    # win rhs rows (partitions 32:45), A half cols 0:256, B half 256:512
    wA1h, wA1l = _split(np.asarray(W_in_A, np.float64)[1])
    wB1h, wB1l = _split(np.asarray(W_in_B, np.float64)[1])
    wA0h, wA0l = _split(np.asarray(W_in_A, np.float64)[0])
    wB0h, wB0l = _split(np.asarray(W_in_B, np.float64)[0])
    bAh, bAl = _split(float(np.asarray(b_out, np.float64)[0]) * np.asarray(W_in_A, np.float64)[1])
    bBh, bBl = _split(float(np.asarray(b_out, np.float64)[1]) * np.asarray(W_in_B, np.float64)[1])
    win = np.zeros((14, 512), np.float32)
    win[0, 0:256] = wA1h       # dAh
    win[1, 256:512] = wB1h     # dBh
    win[2, 0:256] = wA1h       # dAl
    win[3, 256:512] = wB1h     # dBl
    win[4, 0:256] = wA1l       # dAh (dup)
    win[5, 256:512] = wB1l     # dBh (dup)
    win[6, 0:256] = wA0h
    win[7, 0:256] = wA0l
    win[8, 0:256] = bAh
    win[9, 0:256] = bAl
    win[10, 256:512] = wB0h
    win[11, 256:512] = wB0l
    win[12, 256:512] = bBh
    win[13, 256:512] = bBl
    cr[32:46, WIN:WIN + 512] = win
    cf = np.zeros((128, CF_TOT), np.float32)
    cf[:, WOUT + 0:WOUT + 2] = (np.asarray(Wout, np.float64)[0:128] * 0.5).astype(np.float32)
    cf[:, WOUT + 2:WOUT + 4] = (np.asarray(Wout, np.float64)[128:256] * 0.5).astype(np.float32)
    cf[0:64, IDENT:IDENT + 64] = np.eye(64, dtype=np.float32)
    return cr, cf


def prep_core_inputs(settings, nA, nB, Tsteps):
    """Per-core variable arrays. settings [bl, T, 2] int32; nA/nB [bl, T, N] f32."""
    pairs = Tsteps // 2
    bl = settings.shape[0]
    ncat = np.concatenate([nA, nB], axis=2)          # [bl, T, 512]
    # noi[q, p, j*128 + c*32 + b] = ncat[b, 2q+j, 128c+p]
    x = ncat.transpose(1, 2, 0).reshape(Tsteps, 4, 128, bl)     # [t, c, p, b]
    x = x.transpose(0, 2, 1, 3).reshape(pairs, 2, 128, 4 * bl)  # [q, j, p, cb]
    noi = np.ascontiguousarray(x.transpose(0, 2, 1, 3).reshape(pairs, 128, 2 * 4 * bl), np.float32)
    idx = (2 * settings[:, :, 0] + settings[:, :, 1]).astype(np.int64)  # [bl, T]
    oh = np.zeros((4, bl, Tsteps), np.float32)
    for i in range(4):
        oh[i] = (idx.T == i).T.astype(np.float32) if False else (idx == i).astype(np.float32)
    # oh[i, b, t] -> ohp[q, 0:4, j*32+b] (dup at rows 4:8)
    ohp = np.zeros((pairs, 8, 2 * bl), np.float32)
    o = oh.transpose(2, 0, 1).reshape(pairs, 2, 4, bl)   # [q, j, i, b]
    ohp[:, 0:4, :] = o.transpose(0, 2, 1, 3).reshape(pairs, 4, 2 * bl)
    ohp[:, 4:8, :] = ohp[:, 0:4, :]
    # smp rows: bitsA, bitsA, 1, 1, bitsB, bitsB, 1, 1
    bits = settings.astype(np.float32)                   # [bl, T, 2]
    bA = bits[:, :, 0].T.reshape(pairs, 2, bl).reshape(pairs, 2 * bl)
    bB = bits[:, :, 1].T.reshape(pairs, 2, bl).reshape(pairs, 2 * bl)
    smp = np.zeros((pairs, 8, 2 * bl), np.float32)
    smp[:, 0] = bA
    smp[:, 1] = bA
    smp[:, 2] = 1.0
    smp[:, 3] = 1.0
    smp[:, 4] = bB
    smp[:, 5] = bB
    smp[:, 6] = 1.0
    smp[:, 7] = 1.0
    return noi, ohp, smp


_NC_CACHE = {}


def _set_cache_ns(Tsteps, split):
    # The neuron persistent compile cache keys on the HLO fingerprint, which
    # does NOT cover the embedded BIR -> stale-NEFF reuse across kernel edits.
    # Namespace the cache by a hash of this source + build params instead.
    import hashlib
    try:
        src = open(os.path.abspath(__file__), "rb").read()
    except OSError:
        src = b""
    h = hashlib.sha256(src + repr((Tsteps, split)).encode()).hexdigest()[:16]
    path = f"/tmp/neuron-cache-{h}"
    os.environ["NEURON_CC_CACHE_DIR"] = path
    os.environ["NEURON_COMPILE_CACHE_URL"] = path
    # The default cache keys on an HLO fingerprint that excludes the embedded
    # BIR, so kernel edits would silently reuse stale NEFFs.
    import shutil
    shutil.rmtree("/root/.neuron-compile-cache", ignore_errors=True)
    shutil.rmtree(os.path.expanduser("~/.neuron-compile-cache"), ignore_errors=True)


def run_cores(inputs, Tsteps=T, split=True, trace=False):
    """Shard, build/compile (cached), run on 8 cores; return per-core logits [2, pairs*64]."""
    _set_cache_ns(Tsteps, split)
    key = (Tsteps, split)
    if key not in _NC_CACHE:
        _NC_CACHE[key] = build_bass(Tsteps, split)
    nc = _NC_CACHE[key]
    cr, cf = prep_consts(inputs["W_in_A"], inputs["W_rec_A"], inputs["W_in_B"],
                         inputs["W_rec_B"], inputs["Wx"], inputs["b_gru"],
                         inputs["Wout"], inputs["b_out"])
    settings = np.asarray(inputs["settings_seq"])[:, :Tsteps]
    nA = np.asarray(inputs["noise_seq_A"], np.float32)[:, :Tsteps]
    nB = np.asarray(inputs["noise_seq_B"], np.float32)[:, :Tsteps]
    in_maps = []
    for c in range(NCORES):
        sl = slice(c * BL, (c + 1) * BL)
        noi, ohp, smp = prep_core_inputs(settings[sl], nA[sl], nB[sl], Tsteps)
        in_maps.append(dict(cr=cr, cf=cf, noi=noi, ohp=ohp, smp=smp))
    try:
        br = run_bass_kernel_spmd(nc, in_maps, list(range(NCORES)), trace=trace)
    except ModuleNotFoundError:
        br = run_bass_kernel_spmd(nc, in_maps, list(range(NCORES)), trace=False)
    return br


def kernel(settings_seq, noise_seq_A, noise_seq_B,
           W_in_A, W_rec_A, W_in_B, W_rec_B,
           Wx, Wh, b_gru, Wout, b_out,
           T_total, washout_steps, _trace=False, _split=True):
    inputs = dict(settings_seq=np.asarray(settings_seq), noise_seq_A=noise_seq_A,
                  noise_seq_B=noise_seq_B, W_in_A=W_in_A, W_rec_A=W_rec_A,
                  W_in_B=W_in_B, W_rec_B=W_rec_B, Wx=Wx, b_gru=b_gru,
                  Wout=Wout, b_out=b_out)
    Tsteps = int(T_total)
    wash = int(washout_steps)
    br = run_cores(inputs, Tsteps, split=_split, trace=_trace)
    kernel.last_results = br
    b_out = np.asarray(b_out, np.float32)
    pairs = Tsteps // 2
    signs = np.empty((B, Tsteps, 2), np.float32)
    for c in range(NCORES):
        lg = br.results[c]["lgo"].reshape(2, pairs, 2, BL)      # [dim, q, j, b]
        lg = lg.transpose(3, 1, 2, 0).reshape(BL, Tsteps, 2)    # [b, t, dim]
        signs[c * BL:(c + 1) * BL] = np.sign(lg + b_out[None, None, :])
    bits_log = np.asarray(settings_seq, np.int32)[:, :Tsteps].astype(np.float32)
    return (signs[:, wash:, 0], signs[:, wash:, 1], bits_log[:, wash:])
